# revision 8
# baseline (speedup 1.0000x reference)
"""MCR2 loss kernel for Trainium2 — 8 NeuronCores, SPMD.

Strategy
--------
Rows are sorted by class on the host (index bookkeeping only touches the
label vector; the row gather is host-side data staging).  Each core owns
4 of the 32 classes and streams its ~16.5 MiB shard through the tensor
engine, accumulating the 4 class covariances Z_j^T Z_j in PSUM with pure
128x128 fp32 matmuls (no masking).  The total covariance R = sum_j cov_j
is AllReduced across the 8 cores (64 KiB).

The log-dets are computed fully on-device with no transcendentals via a
self-whitening identity: with X ~= R^{-1} (Newton-Schulz, 12 iters),

    loss = sum_j (m_j/2m) tr log(X cov_j * m/m_j)  -  1/2 tr log(X R)

where every logdet(X)/log(d/eps) constant cancels algebraically.  Each
tr log(I+G) is a Taylor series in G whose spectral radius is the
Marchenko-Pastur subsample fluctuation ~2*sqrt(d/m_j) ~= 0.27 regardless
of the population covariance.  Traces of powers tr(G^k) are computed as
Frobenius inner products of PE-built power matrices.
"""

import numpy as np

import concourse.bacc as bacc
import concourse.mybir as mybir
import concourse.tile as tile
from concourse.bass_utils import run_bass_kernel_spmd

M_TOTAL = 262144
D = 128
J = 32
N_CORES = 8
CPC = J // N_CORES  # classes per core
EPS = 0.01
P = 128
K_TAYLOR = 12
NEWTON_ITERS = 15
NMAT = CPC + 1  # 4 class matrices + 1 total
NT = NMAT * K_TAYLOR  # trace slots (60)

_prog_cache = {}

# test-harness knobs: extra kwargs for run_bass_kernel_spmd (e.g. trace=True)
# and the last BassKernelResults (for exec_time_ns / profile inspection).
RUN_KWARGS = {}
LAST_RESULT = None


DEBUG_NO_COLLECTIVE = False


def _build_program(T_CLS, CHUNK):
    f32 = mybir.dt.float32
    add = mybir.AluOpType.add
    mult = mybir.AluOpType.mult
    sub = mybir.AluOpType.subtract
    NCH = T_CLS // CHUNK

    nc = bacc.Bacc(
        "TRN2", target_bir_lowering=False, debug=False, num_devices=N_CORES
    )
    z_dram = nc.dram_tensor("z", [CPC * T_CLS * P, D], f32, kind="ExternalInput")
    consts_dram = nc.dram_tensor("consts", [P, NMAT + 1], f32, kind="ExternalInput")
    ident_dram = nc.dram_tensor("ident", [P, D], f32, kind="ExternalInput")
    out_dram = nc.dram_tensor("trp_out", [P, 64], f32, kind="ExternalOutput")

    with tile.TileContext(nc) as tc:
        with (
            tc.tile_pool(name="const", bufs=1) as constp,
            tc.tile_pool(name="z", bufs=3) as zp,
            tc.tile_pool(name="covps", bufs=1, space="PSUM") as covps,
            tc.tile_pool(name="ps", bufs=1, space="PSUM") as ps,
            tc.tile_pool(name="small", bufs=1) as smallp,
            tc.tile_pool(name="work", bufs=2) as workp,
            tc.tile_pool(name="dram", bufs=1, space="DRAM") as dramp,
        ):
            ident = constp.tile([P, D], f32, tag="ident")
            nc.sync.dma_start(ident[:], ident_dram[:])
            consts = constp.tile([P, NMAT + 1], f32, tag="consts")
            nc.sync.dma_start(consts[:], consts_dram[:])
            twoI = constp.tile([P, D], f32, tag="twoI")
            nc.vector.tensor_scalar_mul(twoI[:], ident[:], 2.0)

            # ---- phase 1: stream shard, accumulate per-class covariances
            zv = z_dram[:].rearrange("(c n p) d -> c p n d", c=CPC, p=P)
            cov_psum = [
                covps.tile([P, D], f32, tag=f"cov{j}", name=f"cov{j}")
                for j in range(CPC)
            ]
            for c in range(CPC):
                for ch in range(NCH):
                    zt = zp.tile([P, CHUNK, D], f32, tag="zchunk")
                    nc.sync.dma_start(
                        zt[:], zv[c, :, ch * CHUNK : (ch + 1) * CHUNK, :]
                    )
                    for t in range(CHUNK):
                        nc.tensor.matmul(
                            cov_psum[c][:],
                            zt[:, t, :],
                            zt[:, t, :],
                            start=(ch == 0 and t == 0),
                            stop=(ch == NCH - 1 and t == CHUNK - 1),
                        )

            # ---- phase 2: covs to SBUF, partial total, AllReduce
            covsb = []
            for c in range(CPC):
                cs = smallp.tile([P, D], f32, tag=f"covsb{c}")
                nc.vector.tensor_copy(cs[:], cov_psum[c][:])
                covsb.append(cs)
            s01 = workp.tile([P, D], f32, tag="s01")
            nc.vector.tensor_tensor(s01[:], covsb[0][:], covsb[1][:], op=add)
            s23 = workp.tile([P, D], f32, tag="s23")
            nc.vector.tensor_tensor(s23[:], covsb[2][:], covsb[3][:], op=add)
            ssum = workp.tile([P, D], f32, tag="ssum")
            nc.vector.tensor_tensor(ssum[:], s01[:], s23[:], op=add)
            ar_in = dramp.tile([P, D], f32, tag="ar_in")
            ar_out = dramp.tile([P, D], f32, tag="ar_out")
            nc.sync.dma_start(ar_in[:], ssum[:])
            if DEBUG_NO_COLLECTIVE:
                nc.sync.dma_start(ar_out[:], ar_in[:])
            else:
                nc.gpsimd.collective_compute(
                    "AllReduce",
                    add,
                    replica_groups=[list(range(N_CORES))],
                    ins=[ar_in.opt()],
                    outs=[ar_out.opt()],
                )
            R = smallp.tile([P, D], f32, tag="R")
            nc.sync.dma_start(R[:], ar_out[:])

            # ---- phase 3: X0 = alpha*I (alpha = 1/tr(R) from host), Newton
            X = workp.tile([P, D], f32, tag="X")
            nc.vector.tensor_scalar_mul(X[:], ident[:], consts[:, NMAT : NMAT + 1])
            for _ in range(NEWTON_ITERS):
                yps = ps.tile([P, D], f32, tag="mm_ps", bufs=3)
                nc.tensor.matmul(yps[:], R[:], X[:], start=True, stop=True)
                tsb = workp.tile([P, D], f32, tag="tsb")
                nc.vector.tensor_tensor(tsb[:], twoI[:], yps[:], op=sub)
                x2ps = ps.tile([P, D], f32, tag="mm_ps", bufs=3)
                nc.tensor.matmul(x2ps[:], X[:], tsb[:], start=True, stop=True)
                Xn = workp.tile([P, D], f32, tag="X")
                nc.vector.tensor_copy(Xn[:], x2ps[:])
                X = Xn

            # ---- phase 4: per-matrix G = s*(X@A) - I, powers, traces
            trp = constp.tile([P, 64], f32, tag="trp")
            nc.vector.memset(trp[:], 0.0)
            mats = covsb + [R]
            for idx in range(NMAT):
                A = mats[idx]
                sc = consts[:, idx : idx + 1]
                g_ps = ps.tile([P, D], f32, tag="mm_ps", bufs=3)
                nc.tensor.matmul(g_ps[:], X[:], A[:], start=True, stop=True)
                gsc = workp.tile([P, D], f32, tag="gsc")
                nc.vector.tensor_scalar_mul(gsc[:], g_ps[:], sc)
                G = workp.tile([P, D], f32, tag="G")
                nc.vector.tensor_tensor(G[:], gsc[:], ident[:], op=sub)
                gt_ps = ps.tile([P, D], f32, tag="mm_ps", bufs=3)
                nc.tensor.matmul(gt_ps[:], A[:], X[:], start=True, stop=True)
                gtsc = workp.tile([P, D], f32, tag="gsc")
                nc.vector.tensor_scalar_mul(gtsc[:], gt_ps[:], sc)
                GT = workp.tile([P, D], f32, tag="GT")
                nc.vector.tensor_tensor(GT[:], gtsc[:], ident[:], op=sub)
                # powers up to K_TAYLOR//2 (and their transposes)
                powers = [(G, GT)]
                for k in range(2, K_TAYLOR // 2 + 1):
                    Pprev, PprevT = powers[-1]
                    pk_ps = ps.tile([P, D], f32, tag="mm_ps", bufs=3)
                    nc.tensor.matmul(pk_ps[:], PprevT[:], G[:], start=True, stop=True)
                    Pk = workp.tile([P, D], f32, tag=f"P{k}")
                    nc.vector.tensor_copy(Pk[:], pk_ps[:])
                    pkt_ps = ps.tile([P, D], f32, tag="mm_ps", bufs=3)
                    nc.tensor.matmul(pkt_ps[:], Pprev[:], GT[:], start=True, stop=True)
                    PkT = workp.tile([P, D], f32, tag=f"P{k}T")
                    nc.vector.tensor_copy(PkT[:], pkt_ps[:])
                    powers.append((Pk, PkT))
                # trace pairs in k order: tr(G^k) = <L, R>_F
                pairs = [(ident, G)]
                H = K_TAYLOR // 2
                for i in range(1, H + 1):
                    PiT = powers[i - 1][1]
                    pairs.append((PiT, powers[i - 1][0]))  # k = 2i
                    if i < H:
                        pairs.append((PiT, powers[i][0]))  # k = 2i + 1
                assert len(pairs) == K_TAYLOR
                for kk, (L, Rm) in enumerate(pairs):
                    scr2 = workp.tile([P, D], f32, tag="ttr_scr")
                    slot = idx * K_TAYLOR + kk
                    nc.vector.tensor_tensor(scr2[:], L[:], Rm[:], op=mult)
                    nc.vector.tensor_reduce(
                        trp[:, slot : slot + 1], scr2[:],
                        axis=mybir.AxisListType.X, op=add,
                    )

            # ---- phase 5: ship trace partials; host applies coefficients
            nc.sync.dma_start(out_dram[:], trp[:])

    nc.compile()
    return nc


def _get_program(T_CLS, CHUNK):
    key = (T_CLS, CHUNK)
    if key not in _prog_cache:
        _prog_cache[key] = _build_program(T_CLS, CHUNK)
    return _prog_cache[key]


def _host_reference(Z, lab):
    """Numpy fp64 fallback for degenerate inputs (tiny classes)."""
    Z = Z.astype(np.float64)
    m, d = Z.shape
    c = d / (m * EPS)
    sgn, ld = np.linalg.slogdet(c * (Z.T @ Z))
    loss = -sgn * ld / 2.0
    for j in range(J):
        mask = lab == j
        mj = int(mask.sum())
        if mj == 0:
            continue
        cj = d / (max(mj, 1) * EPS)
        sgn, ldj = np.linalg.slogdet(cj * (Z[mask].T @ Z[mask]))
        loss += sgn * ldj * mj / (2.0 * m)
    return np.asarray(np.float32(loss))


def kernel(cls_score, label):
    Z = np.ascontiguousarray(np.asarray(cls_score), dtype=np.float32)
    lab = np.asarray(label).astype(np.int64, copy=False).ravel()
    m, d = Z.shape
    assert (m, d) == (M_TOTAL, D), f"unexpected shape {Z.shape}"
    counts = np.bincount(lab, minlength=J)[:J]

    nonempty = counts[counts > 0]
    if len(nonempty) == 0 or nonempty.min() < 2048:
        return _host_reference(Z, lab)

    # tiles per class, padded to a multiple of 6 so CHUNK = T_CLS // 6
    maxc = int(counts.max())
    T_CLS = 6 * ((maxc + 6 * P - 1) // (6 * P))
    CHUNK = T_CLS // 6

    order = np.argsort(lab, kind="stable")
    Zs = Z[order]
    bounds = np.zeros(J + 1, np.int64)
    bounds[1:] = np.cumsum(counts)

    I128 = np.eye(P, dtype=np.float32)
    # alpha = 1/tr(Z^T Z): safe Newton seed since tr >= lambda_max
    alpha_h = np.float32(1.0 / float(np.einsum("ij,ij->", Z, Z, dtype=np.float64)))
    rows_pc = CPC * T_CLS * P
    in_maps = []
    coeffs_per_core = []
    for c in range(N_CORES):
        zc = np.zeros((rows_pc, D), np.float32)
        consts = np.zeros((P, NMAT + 1), np.float32)
        consts[:, CPC] = 1.0
        consts[:, NMAT] = alpha_h
        coeffs = np.zeros(64, np.float64)
        for s in range(CPC):
            j = c * CPC + s
            mj = int(counts[j])
            if mj == 0:
                continue  # consts/coeffs stay 0 -> no contribution
            zc[s * T_CLS * P : s * T_CLS * P + mj] = Zs[bounds[j] : bounds[j + 1]]
            consts[:, s] = M_TOTAL / mj
            w = mj / (2.0 * M_TOTAL)
            for k in range(1, K_TAYLOR + 1):
                coeffs[s * K_TAYLOR + k - 1] = w * ((-1.0) ** (k + 1)) / k
        wtot = -1.0 / (2.0 * N_CORES)
        for k in range(1, K_TAYLOR + 1):
            coeffs[CPC * K_TAYLOR + k - 1] = wtot * ((-1.0) ** (k + 1)) / k
        coeffs_per_core.append(coeffs)
        in_maps.append({"z": zc, "consts": consts, "ident": I128})

    nc = _get_program(T_CLS, CHUNK)
    res = run_bass_kernel_spmd(nc, in_maps, list(range(N_CORES)), **RUN_KWARGS)
    global LAST_RESULT
    LAST_RESULT = res
    total = 0.0
    for c, r in enumerate(res.results):
        tsums = r["trp_out"].astype(np.float64).sum(axis=0)  # [64]
        total += float(tsums @ coeffs_per_core[c])
    return np.asarray(np.float32(total))


# revision 10
# speedup vs baseline: 1.0239x; 1.0239x over previous
"""MCR2 loss kernel for Trainium2 — 8 NeuronCores, SPMD.

Strategy
--------
Rows are sorted by class on the host (index bookkeeping only touches the
label vector; the row gather is host-side data staging).  Each core owns
4 of the 32 classes and streams its ~16.5 MiB shard through the tensor
engine, accumulating the 4 class covariances Z_j^T Z_j in PSUM with pure
128x128 fp32 matmuls (no masking).  The total covariance R = sum_j cov_j
is AllReduced across the 8 cores (64 KiB).

The log-dets are computed fully on-device with no transcendentals via a
self-whitening identity: with X ~= R^{-1} (Newton-Schulz, 12 iters),

    loss = sum_j (m_j/2m) tr log(X cov_j * m/m_j)  -  1/2 tr log(X R)

where every logdet(X)/log(d/eps) constant cancels algebraically.  Each
tr log(I+G) is a Taylor series in G whose spectral radius is the
Marchenko-Pastur subsample fluctuation ~2*sqrt(d/m_j) ~= 0.27 regardless
of the population covariance.  Traces of powers tr(G^k) are computed as
Frobenius inner products of PE-built power matrices.
"""

import numpy as np

import concourse.bacc as bacc
import concourse.mybir as mybir
import concourse.tile as tile
from concourse.bass_utils import run_bass_kernel_spmd

M_TOTAL = 262144
D = 128
J = 32
N_CORES = 8
CPC = J // N_CORES  # classes per core
EPS = 0.01
P = 128
K_TAYLOR = 12
NEWTON_ITERS = 15
NMAT = CPC + 1  # 4 class matrices + 1 total
NT = NMAT * K_TAYLOR  # trace slots (60)

_prog_cache = {}

# test-harness knobs: extra kwargs for run_bass_kernel_spmd (e.g. trace=True)
# and the last BassKernelResults (for exec_time_ns / profile inspection).
RUN_KWARGS = {}
LAST_RESULT = None


DEBUG_NO_COLLECTIVE = False


def _build_program(T_CLS, CHUNK):
    f32 = mybir.dt.float32
    bf16 = mybir.dt.bfloat16
    add = mybir.AluOpType.add
    mult = mybir.AluOpType.mult
    sub = mybir.AluOpType.subtract
    NCH = T_CLS // CHUNK

    nc = bacc.Bacc(
        "TRN2", target_bir_lowering=False, debug=False, num_devices=N_CORES
    )
    z_dram = nc.dram_tensor("z", [CPC * T_CLS * P, D], f32, kind="ExternalInput")
    consts_dram = nc.dram_tensor("consts", [P, NMAT + 1], f32, kind="ExternalInput")
    ident_dram = nc.dram_tensor("ident", [P, D], f32, kind="ExternalInput")
    out_dram = nc.dram_tensor("trp_out", [P, 64], f32, kind="ExternalOutput")

    with tile.TileContext(nc) as tc:
        with (
            tc.tile_pool(name="const", bufs=1) as constp,
            tc.tile_pool(name="z", bufs=3) as zp,
            tc.tile_pool(name="covps", bufs=1, space="PSUM") as covps,
            tc.tile_pool(name="ps", bufs=1, space="PSUM") as ps,
            tc.tile_pool(name="small", bufs=1) as smallp,
            tc.tile_pool(name="work", bufs=2) as workp,
            tc.tile_pool(name="dram", bufs=1, space="DRAM") as dramp,
        ):
            ident = constp.tile([P, D], f32, tag="ident")
            nc.sync.dma_start(ident[:], ident_dram[:])
            consts = constp.tile([P, NMAT + 1], f32, tag="consts")
            nc.sync.dma_start(consts[:], consts_dram[:])
            twoI = constp.tile([P, D], f32, tag="twoI")
            nc.vector.tensor_scalar_mul(twoI[:], ident[:], 2.0)
            ident_bf = constp.tile([P, D], bf16, tag="ident_bf")
            nc.vector.tensor_copy(ident_bf[:], ident[:])
            warm = constp.tile([P, D], bf16, tag="warm")
            nc.vector.memset(warm[:], 0.0)
            for _ in range(48):
                wps = ps.tile([P, D], f32, tag="mm_ps", bufs=3)
                nc.tensor.matmul(wps[:], warm[:], warm[:], start=True, stop=True)

            # ---- phase 1: stream shard, accumulate per-class covariances
            zv = z_dram[:].rearrange("(c n p) d -> c p n d", c=CPC, p=P)
            cov_psum = [
                covps.tile([P, D], f32, tag=f"cov{j}", name=f"cov{j}")
                for j in range(CPC)
            ]
            for c in range(CPC):
                for ch in range(NCH):
                    zt = zp.tile([P, CHUNK, D], f32, tag="zchunk")
                    nc.sync.dma_start(
                        zt[:], zv[c, :, ch * CHUNK : (ch + 1) * CHUNK, :]
                    )
                    ztb = zp.tile([P, CHUNK, D], bf16, tag="zchunkb")
                    nc.vector.tensor_copy(ztb[:], zt[:])
                    for t in range(CHUNK):
                        nc.tensor.matmul(
                            cov_psum[c][:],
                            ztb[:, t, :],
                            ztb[:, t, :],
                            start=(ch == 0 and t == 0),
                            stop=(ch == NCH - 1 and t == CHUNK - 1),
                        )

            # ---- phase 2: covs to SBUF, partial total, AllReduce
            covsb = []
            for c in range(CPC):
                cs = smallp.tile([P, D], f32, tag=f"covsb{c}")
                nc.vector.tensor_copy(cs[:], cov_psum[c][:])
                covsb.append(cs)
            s01 = workp.tile([P, D], f32, tag="s01")
            nc.vector.tensor_tensor(s01[:], covsb[0][:], covsb[1][:], op=add)
            s23 = workp.tile([P, D], f32, tag="s23")
            nc.vector.tensor_tensor(s23[:], covsb[2][:], covsb[3][:], op=add)
            ssum = workp.tile([P, D], f32, tag="ssum")
            nc.vector.tensor_tensor(ssum[:], s01[:], s23[:], op=add)
            ar_in = dramp.tile([P, D], f32, tag="ar_in")
            ar_out = dramp.tile([P, D], f32, tag="ar_out")
            nc.sync.dma_start(ar_in[:], ssum[:])
            if DEBUG_NO_COLLECTIVE:
                nc.sync.dma_start(ar_out[:], ar_in[:])
            else:
                nc.gpsimd.collective_compute(
                    "AllReduce",
                    add,
                    replica_groups=[list(range(N_CORES))],
                    ins=[ar_in.opt()],
                    outs=[ar_out.opt()],
                )
            R = smallp.tile([P, D], f32, tag="R")
            nc.sync.dma_start(R[:], ar_out[:])
            R_bf = smallp.tile([P, D], bf16, tag="R_bf")
            nc.vector.tensor_copy(R_bf[:], R[:])

            # ---- phase 3: X0 = alpha*I (alpha = 1/tr(R) from host), Newton
            X = workp.tile([P, D], bf16, tag="X")
            nc.vector.tensor_scalar_mul(X[:], ident[:], consts[:, NMAT : NMAT + 1])
            for _ in range(NEWTON_ITERS - 2):
                yps = ps.tile([P, D], f32, tag="mm_ps", bufs=3)
                nc.tensor.matmul(yps[:], R_bf[:], X[:], start=True, stop=True)
                tsb = workp.tile([P, D], bf16, tag="tsb")
                nc.vector.tensor_tensor(tsb[:], twoI[:], yps[:], op=sub)
                x2ps = ps.tile([P, D], f32, tag="mm_ps", bufs=3)
                nc.tensor.matmul(x2ps[:], X[:], tsb[:], start=True, stop=True)
                Xn = workp.tile([P, D], bf16, tag="X")
                nc.vector.tensor_copy(Xn[:], x2ps[:])
                X = Xn
            Xf = workp.tile([P, D], f32, tag="Xf")
            nc.vector.tensor_copy(Xf[:], X[:])
            for _ in range(2):
                yps = ps.tile([P, D], f32, tag="mm_ps", bufs=3)
                nc.tensor.matmul(yps[:], R[:], Xf[:], start=True, stop=True)
                tsb32 = workp.tile([P, D], f32, tag="tsb32")
                nc.vector.tensor_tensor(tsb32[:], twoI[:], yps[:], op=sub)
                x2ps = ps.tile([P, D], f32, tag="mm_ps", bufs=3)
                nc.tensor.matmul(x2ps[:], Xf[:], tsb32[:], start=True, stop=True)
                Xn2 = workp.tile([P, D], f32, tag="Xf")
                nc.vector.tensor_copy(Xn2[:], x2ps[:])
                Xf = Xn2

            # ---- phase 4: per-matrix G = s*(X@A) - I, powers, traces
            trp = constp.tile([P, 64], f32, tag="trp")
            nc.vector.memset(trp[:], 0.0)
            mats = covsb + [R]
            for idx in range(NMAT):
                A = mats[idx]
                sc = consts[:, idx : idx + 1]
                g_ps = ps.tile([P, D], f32, tag="mm_ps", bufs=3)
                nc.tensor.matmul(g_ps[:], Xf[:], A[:], start=True, stop=True)
                gsc = workp.tile([P, D], f32, tag="gsc")
                nc.vector.tensor_scalar_mul(gsc[:], g_ps[:], sc)
                G = workp.tile([P, D], bf16, tag="G")
                nc.vector.tensor_tensor(G[:], gsc[:], ident[:], op=sub)
                gt_ps = ps.tile([P, D], f32, tag="mm_ps", bufs=3)
                nc.tensor.matmul(gt_ps[:], A[:], Xf[:], start=True, stop=True)
                gtsc = workp.tile([P, D], f32, tag="gsc")
                nc.vector.tensor_scalar_mul(gtsc[:], gt_ps[:], sc)
                GT = workp.tile([P, D], bf16, tag="GT")
                nc.vector.tensor_tensor(GT[:], gtsc[:], ident[:], op=sub)
                # powers up to K_TAYLOR//2 (and their transposes)
                powers = [(G, GT)]
                for k in range(2, K_TAYLOR // 2 + 1):
                    Pprev, PprevT = powers[-1]
                    pk_ps = ps.tile([P, D], f32, tag="mm_ps", bufs=3)
                    nc.tensor.matmul(pk_ps[:], PprevT[:], G[:], start=True, stop=True)
                    Pk = workp.tile([P, D], bf16, tag=f"P{k}")
                    nc.vector.tensor_copy(Pk[:], pk_ps[:])
                    pkt_ps = ps.tile([P, D], f32, tag="mm_ps", bufs=3)
                    nc.tensor.matmul(pkt_ps[:], Pprev[:], GT[:], start=True, stop=True)
                    PkT = workp.tile([P, D], bf16, tag=f"P{k}T")
                    nc.vector.tensor_copy(PkT[:], pkt_ps[:])
                    powers.append((Pk, PkT))
                # trace pairs in k order: tr(G^k) = <L, R>_F
                pairs = [(ident_bf, G)]
                H = K_TAYLOR // 2
                for i in range(1, H + 1):
                    PiT = powers[i - 1][1]
                    pairs.append((PiT, powers[i - 1][0]))  # k = 2i
                    if i < H:
                        pairs.append((PiT, powers[i][0]))  # k = 2i + 1
                assert len(pairs) == K_TAYLOR
                for kk, (L, Rm) in enumerate(pairs):
                    scr2 = workp.tile([P, D], f32, tag="ttr_scr")
                    slot = idx * K_TAYLOR + kk
                    nc.vector.tensor_tensor(scr2[:], L[:], Rm[:], op=mult)
                    nc.vector.tensor_reduce(
                        trp[:, slot : slot + 1], scr2[:],
                        axis=mybir.AxisListType.X, op=add,
                    )

            # ---- phase 5: ship trace partials; host applies coefficients
            nc.sync.dma_start(out_dram[:], trp[:])

    nc.compile()
    return nc


def _get_program(T_CLS, CHUNK):
    key = (T_CLS, CHUNK)
    if key not in _prog_cache:
        _prog_cache[key] = _build_program(T_CLS, CHUNK)
    return _prog_cache[key]


def _host_reference(Z, lab):
    """Numpy fp64 fallback for degenerate inputs (tiny classes)."""
    Z = Z.astype(np.float64)
    m, d = Z.shape
    c = d / (m * EPS)
    sgn, ld = np.linalg.slogdet(c * (Z.T @ Z))
    loss = -sgn * ld / 2.0
    for j in range(J):
        mask = lab == j
        mj = int(mask.sum())
        if mj == 0:
            continue
        cj = d / (max(mj, 1) * EPS)
        sgn, ldj = np.linalg.slogdet(cj * (Z[mask].T @ Z[mask]))
        loss += sgn * ldj * mj / (2.0 * m)
    return np.asarray(np.float32(loss))


def kernel(cls_score, label):
    Z = np.ascontiguousarray(np.asarray(cls_score), dtype=np.float32)
    lab = np.asarray(label).astype(np.int64, copy=False).ravel()
    m, d = Z.shape
    assert (m, d) == (M_TOTAL, D), f"unexpected shape {Z.shape}"
    counts = np.bincount(lab, minlength=J)[:J]

    nonempty = counts[counts > 0]
    if len(nonempty) == 0 or nonempty.min() < 2048:
        return _host_reference(Z, lab)

    # tiles per class, padded to a multiple of 6 so CHUNK = T_CLS // 6
    maxc = int(counts.max())
    T_CLS = 6 * ((maxc + 6 * P - 1) // (6 * P))
    CHUNK = T_CLS // 6

    order = np.argsort(lab, kind="stable")
    Zs = Z[order]
    bounds = np.zeros(J + 1, np.int64)
    bounds[1:] = np.cumsum(counts)

    I128 = np.eye(P, dtype=np.float32)
    # alpha = 1/tr(Z^T Z): safe Newton seed since tr >= lambda_max
    alpha_h = np.float32(1.0 / float(np.einsum("ij,ij->", Z, Z, dtype=np.float64)))
    rows_pc = CPC * T_CLS * P
    in_maps = []
    coeffs_per_core = []
    for c in range(N_CORES):
        zc = np.zeros((rows_pc, D), np.float32)
        consts = np.zeros((P, NMAT + 1), np.float32)
        consts[:, CPC] = 1.0
        consts[:, NMAT] = alpha_h
        coeffs = np.zeros(64, np.float64)
        for s in range(CPC):
            j = c * CPC + s
            mj = int(counts[j])
            if mj == 0:
                continue  # consts/coeffs stay 0 -> no contribution
            zc[s * T_CLS * P : s * T_CLS * P + mj] = Zs[bounds[j] : bounds[j + 1]]
            consts[:, s] = M_TOTAL / mj
            w = mj / (2.0 * M_TOTAL)
            for k in range(1, K_TAYLOR + 1):
                coeffs[s * K_TAYLOR + k - 1] = w * ((-1.0) ** (k + 1)) / k
        wtot = -1.0 / (2.0 * N_CORES)
        for k in range(1, K_TAYLOR + 1):
            coeffs[CPC * K_TAYLOR + k - 1] = wtot * ((-1.0) ** (k + 1)) / k
        coeffs_per_core.append(coeffs)
        in_maps.append({"z": zc, "consts": consts, "ident": I128})

    nc = _get_program(T_CLS, CHUNK)
    res = run_bass_kernel_spmd(nc, in_maps, list(range(N_CORES)), **RUN_KWARGS)
    global LAST_RESULT
    LAST_RESULT = res
    total = 0.0
    for c, r in enumerate(res.results):
        tsums = r["trp_out"].astype(np.float64).sum(axis=0)  # [64]
        total += float(tsums @ coeffs_per_core[c])
    return np.asarray(np.float32(total))
